# revision 18
# baseline (speedup 1.0000x reference)
"""Distributed Trainium2 kernel for DeepseekV3-style GQA attention.

Problem (hardcoded): B=1, S=4096, H=2048, NQ=16 q heads, NKV=4 kv heads,
D=128, rotate-half RoPE (theta=10000) over full head dim, causal softmax,
o_proj. 8 NeuronCores, tensor-parallel over heads:

  core c: q heads {2c, 2c+1}, kv head c//2 (replicated across the pair),
  Wq/Wk/Wv column-sharded, attention computed flash-style in bf16 with
  f32 PSUM accumulation, attention output produced transposed [j, s],
  AllGathered in 4 sequence chunks (overlapped with attention compute),
  o_proj column-sharded (Wo[:, c*256:(c+1)*256]) -> outT shard [256, S].

Host side: casts inputs to bf16, pre-transposes x, builds cos/sin tables
from position_ids, shards weights, transposes + concatenates out shards.
"""
import os
import sys

sys.path.insert(0, "/opt/trn_rl_repo")

import numpy as np
import ml_dtypes

import concourse.bass as bass
import concourse.bacc as bacc
import concourse.mybir as mybir
import concourse.tile as tile
from concourse.bass_utils import run_bass_kernel_spmd

BF16 = mybir.dt.bfloat16
F32 = mybir.dt.float32
NPBF16 = ml_dtypes.bfloat16

B, S, H = 1, 4096, 2048
NQ, NKV, D = 16, 4, 128
THETA = 10000.0
NCORES = 8
HPC = NQ // NCORES          # q heads per core = 2
OC = H // NCORES            # o_proj out cols per core = 256
SC = 512                    # projection s-chunk
NSC = S // SC               # 8
NKT = S // 128              # 32 k tiles of 128
QS = 512                    # attention q supertile
NQS = S // QS               # 8
CHUNK = 1024                # allgather s-chunk
NCH = S // CHUNK            # 4
SCALE = 1.0 / float(np.sqrt(D))

_cached = {}


def _build():
    nc = bacc.Bacc("TRN2", target_bir_lowering=False, debug=False,
                   num_devices=NCORES)

    # all weight/activation params arrive pre-shuffled into SBUF tile layout
    # [128, 16*cols] so loads are one contiguous descriptor per partition
    xT = nc.declare_dram_parameter("xT", [NSC, 128, 16 * SC], BF16, isOutput=False)
    wq = nc.declare_dram_parameter("wq", [128, 16 * HPC * D], BF16, isOutput=False)
    wk = nc.declare_dram_parameter("wk", [128, 16 * D], BF16, isOutput=False)
    wv = nc.declare_dram_parameter("wv", [128, 16 * D], BF16, isOutput=False)
    wo = nc.declare_dram_parameter("wo", [128, 16 * H], BF16, isOutput=False)
    cosT = nc.declare_dram_parameter("cosT", [D, S], BF16, isOutput=False)
    sinT = nc.declare_dram_parameter("sinT", [D, S], BF16, isOutput=False)
    trimask = nc.declare_dram_parameter("trimask", [128, 128], BF16, isOutput=False)
    identity = nc.declare_dram_parameter("identity", [128, 128], BF16, isOutput=False)
    out = nc.declare_dram_parameter("out", [NCH * 128, H], F32, isOutput=True)

    with tile.TileContext(nc) as tc:
        with (
            tc.tile_pool(name="const", bufs=1) as constp,
            tc.tile_pool(name="persist", bufs=1) as persist,
            tc.tile_pool(name="xtp", bufs=2) as xtp,
            tc.tile_pool(name="ropep", bufs=2) as ropep,
            tc.tile_pool(name="ptp", bufs=3) as ptp,
            tc.tile_pool(name="attnp", bufs=3) as attnp,
            tc.tile_pool(name="smallp", bufs=3) as smallp,
            tc.tile_pool(name="agp", bufs=2) as agp,
            tc.tile_pool(name="outp", bufs=1) as outp,
            tc.tile_pool(name="dram", bufs=1, space="DRAM") as dramp,
        ):
            # ---- weights first (first matmuls need wq + xt chunk 0) ----
            wq_all = persist.tile([128, 16 * HPC * D], BF16, tag="wq")
            nc.sync.dma_start(wq_all[:], wq[:])
            wk_all = persist.tile([128, 16 * D], BF16, tag="wk")
            nc.sync.dma_start(wk_all[:], wk[:])
            wv_all = persist.tile([128, 16 * D], BF16, tag="wv")
            nc.sync.dma_start(wv_all[:], wv[:])

            cos_sb = persist.tile([128, S], BF16, tag="cos")
            sin_sb = persist.tile([128, S], BF16, tag="sin")
            tri = constp.tile([128, 128], BF16, tag="tri")
            ident = constp.tile([128, 128], BF16, tag="ident")
            wo_all = persist.tile([128, 16 * H], BF16, tag="wo")

            def wqt(t, h):
                return wq_all[:, t * HPC * D + h * D: t * HPC * D + (h + 1) * D]

            QT = [persist.tile([128, S], BF16, tag=f"qt{h}", name=f"qt{h}")
                  for h in range(HPC)]
            KT = persist.tile([128, S], BF16, tag="kt")
            V = []
            for kt in range(NKT):
                vt = persist.tile([128, D + 1], BF16, tag=f"v{kt}")
                nc.gpsimd.memset(vt[:, D:D + 1], 1.0)
                V.append(vt)

            # tiny barrier collective: absorbs the ~50us core-launch skew
            # during the input-DMA phase so the first real A2A doesn't pay it
            bar_in = dramp.tile([1, 128], F32, tag="bar_in", name="bar_in")
            bar_out = dramp.tile([1, 128], F32, tag="bar_out", name="bar_out",
                                 addr_space="Shared")
            nc.gpsimd.collective_compute(
                "AllReduce", mybir.AluOpType.add,
                replica_groups=[list(range(NCORES))],
                ins=[bar_in.opt()], outs=[bar_out.opt()])

            # ---- fused: projections + RoPE + attention + A2A + o_proj ----
            with (
                tc.tile_pool(name="bigps", bufs=2, space="PSUM") as bigps,
                tc.tile_pool(name="attps", bufs=4, space="PSUM") as attps,
                tc.tile_pool(name="smps", bufs=2, space="PSUM") as smps,
            ):
                attnT_cur = [None, None]
                bounces = []

                def _attv(kt, j, q_lo, pt, att, qs):
                    for qsub in range(max(j, 0), 4):
                        nc.tensor.matmul(
                            att[qsub][:],
                            pt[:, qsub * 128 - q_lo:qsub * 128 - q_lo + 128],
                            V[kt][:],
                            start=(kt == 0), stop=(kt == 4 * qs + qsub))

                def attention(head, qs):
                    q_off = qs * QS
                    att = [attps.tile([128, D + 1], F32, tag="att", name=f"att{i}")
                           for i in range(4)]
                    nkt = 4 * qs + 4
                    pend = None  # (kt, j, q_lo, pt)
                    for kt in range(nkt):
                        j = kt - 4 * qs
                        q_lo = 128 * j if j > 0 else 0
                        N = QS - q_lo
                        st_ps = bigps.tile([128, QS], F32, tag="big", name="st_ps")
                        nc.tensor.matmul(
                            st_ps[:, 0:N],
                            KT[:, kt * 128:(kt + 1) * 128],
                            QT[head][:, q_off + q_lo:q_off + QS],
                            start=True, stop=True)
                        pt = ptp.tile([128, QS], BF16, tag="pt", name="pt")
                        nc.scalar.activation(pt[:, 0:N], st_ps[:, 0:N],
                                             mybir.ActivationFunctionType.Exp,
                                             scale=SCALE)
                        if j >= 0:
                            nc.vector.tensor_tensor(pt[:, 0:128], pt[:, 0:128],
                                                    tri[:],
                                                    mybir.AluOpType.mult)
                        if pend is not None:
                            _attv(*pend, att, qs)
                        pend = (kt, j, q_lo, pt)
                    _attv(*pend, att, qs)

                    # normalize + transpose into attnT
                    for qsub in range(4):
                        recip = smallp.tile([128, 1], F32, tag="recip",
                                            name="recip")
                        nc.vector.reciprocal(recip[:], att[qsub][:, D:D + 1])
                        attn_n = smallp.tile([128, 128], BF16, tag="attn_n",
                                             name="attn_n")
                        nc.vector.tensor_scalar(attn_n[:], att[qsub][:, 0:D],
                                                recip[:], None,
                                                mybir.AluOpType.mult)
                        tr = smps.tile([128, 128], BF16, tag="tr", name="tr")
                        nc.tensor.transpose(tr[:], attn_n[:], ident[:])
                        col = (qs % 2) * QS + qsub * 128
                        nc.vector.tensor_copy(
                            attnT_cur[head][:, col:col + 128], tr[:])

                def emit_a2a(ci, bounce):
                    ex = dramp.tile([NCORES * 2 * 128, 128], BF16,
                                    tag=f"a2a{ci}", name=f"a2a{ci}")
                    nc.gpsimd.collective_compute(
                        "AllToAll", mybir.AluOpType.bypass,
                        replica_groups=[list(range(NCORES))],
                        ins=[bounce.opt()], outs=[ex.opt()])
                    bounces.append(ex)

                def emit_oproj(ci):
                    ex = bounces[ci]
                    ag_all = agp.tile([128, 16 * 128], BF16, tag="ag",
                                      name="ag_all")
                    nc.scalar.dma_start(
                        ag_all[:].rearrange("p (t s) -> p t s", t=16),
                        ex[:].rearrange("(t p) s -> p t s", p=128))
                    o_sb = outp.tile([128, H], F32, tag="osb", name="o_sb")
                    for ocg in range(H // 512):
                        o_ps = smps.tile([128, 512], F32, tag="tr",
                                         name="o_ps")
                        for jt in range(16):
                            nc.tensor.matmul(
                                o_ps[:],
                                ag_all[:, jt * 128:(jt + 1) * 128],
                                wo_all[:, jt * H + ocg * 512:
                                       jt * H + (ocg + 1) * 512],
                                start=(jt == 0), stop=(jt == 15))
                        nc.vector.tensor_copy(
                            o_sb[:, ocg * 512:(ocg + 1) * 512], o_ps[:])
                    nc.scalar.dma_start(out[ci * 128:(ci + 1) * 128, :],
                                        o_sb[:])

                bounce_cur = None
                for sc in range(NSC):
                    off = sc * SC
                    xt_all = xtp.tile([128, 16 * SC], BF16, tag="xt",
                                      name="xt_all")
                    nc.sync.dma_start(xt_all[:], xT[sc])
                    if sc == 0:
                        nc.sync.dma_start(cos_sb[:], cosT[:])
                        nc.sync.dma_start(sin_sb[:], sinT[:])
                        nc.sync.dma_start(tri[:], trimask[:])
                        nc.sync.dma_start(ident[:], identity[:])
                    if sc == 1:
                        # wo is first needed at oproj(0); late load keeps it
                        # off the startup critical path
                        nc.scalar.dma_start(wo_all[:], wo[:])

                    def xts(t):
                        return xt_all[:, t * SC:(t + 1) * SC]

                    # q/k projections -> transposed layout [d, s]
                    def project_rope(lhs_of_t, dst):
                        ps = bigps.tile([128, SC], F32, tag="big", name="ps")
                        for t in range(16):
                            nc.tensor.matmul(ps[:], lhs_of_t(t), xts(t),
                                             start=(t == 0), stop=(t == 15))
                        raw = ropep.tile([128, SC], BF16, tag="raw", name="raw")
                        nc.vector.tensor_copy(raw[:], ps[:])
                        sw = ropep.tile([128, SC], BF16, tag="sw", name="sw")
                        nc.sync.dma_start(sw[0:64, :], raw[64:128, :])
                        nc.sync.dma_start(sw[64:128, :], raw[0:64, :])
                        t1 = ropep.tile([128, SC], BF16, tag="t1", name="t1")
                        nc.vector.tensor_tensor(t1[:], raw[:],
                                                cos_sb[:, off:off + SC],
                                                mybir.AluOpType.mult)
                        t2 = ropep.tile([128, SC], BF16, tag="t2", name="t2")
                        nc.vector.tensor_tensor(t2[:], sw[:],
                                                sin_sb[:, off:off + SC],
                                                mybir.AluOpType.mult)
                        nc.vector.tensor_tensor(dst[:, off:off + SC], t1[:],
                                                t2[:], mybir.AluOpType.add)

                    for h in range(HPC):
                        project_rope(lambda t, h=h: wqt(t, h), QT[h])
                    project_rope(lambda t: wk_all[:, t * D:(t + 1) * D], KT)

                    # v projection (natural [s, d] layout)
                    for st in range(SC // 128):
                        v_ps = smps.tile([128, 128], F32, tag="tr", name="v_ps")
                        for t in range(16):
                            nc.tensor.matmul(
                                v_ps[:], xt_all[:, t * SC + st * 128:
                                                t * SC + (st + 1) * 128],
                                wv_all[:, t * D:(t + 1) * D],
                                start=(t == 0), stop=(t == 15))
                        kti = sc * (SC // 128) + st
                        nc.vector.tensor_copy(V[kti][:, 0:D], v_ps[:])

                    # attention for qs == sc (all needed K/V tiles now exist)
                    qs = sc
                    ci = qs // 2
                    if qs % 2 == 0:
                        bounce_cur = dramp.tile([NCORES * 2 * 128, 128], BF16,
                                                tag=f"bn{ci}", name=f"bn{ci}")
                        for head in range(HPC):
                            attnT_cur[head] = attnp.tile(
                                [128, CHUNK], BF16, tag=f"attnT{head}",
                                name=f"attnT{head}_{qs}")
                    if qs == 7:
                        emit_oproj(2)
                    for head in range(HPC):
                        attention(head, qs)
                    if qs % 2 == 1:
                        bv = bounce_cur[:].rearrange(
                            "(d h j) s -> h j d s", d=NCORES, h=HPC)
                        for head in range(HPC):
                            nc.scalar.dma_start(
                                bv[head],
                                attnT_cur[head][:].rearrange(
                                    "j (d s) -> j d s", d=NCORES))
                        emit_a2a(ci, bounce_cur)
                    if qs == 4:
                        emit_oproj(0)
                    elif qs == 6:
                        emit_oproj(1)
                emit_oproj(NCH - 1)

    nc.compile()
    return nc


def _get_nc():
    if "nc" not in _cached:
        _cached["nc"] = _build()
    return _cached["nc"]


def _prep_inputs(hidden_states, Wq, Wk, Wv, Wo, position_ids):
    x = np.asarray(hidden_states, dtype=np.float32).reshape(S, H)
    # [sc, p, t*512+s] layout: contiguous per-partition rows for the DMA
    xT = np.ascontiguousarray(
        x.T.reshape(16, 128, NSC, SC).transpose(2, 1, 0, 3)
        .reshape(NSC, 128, 16 * SC)).astype(NPBF16)

    def wshuf(W):
        n = W.shape[1]
        return np.ascontiguousarray(
            W.reshape(16, 128, n).transpose(1, 0, 2).reshape(128, 16 * n)
        ).astype(NPBF16)
    Wq = np.asarray(Wq, dtype=np.float32)
    Wk = np.asarray(Wk, dtype=np.float32)
    Wv = np.asarray(Wv, dtype=np.float32)
    Wo = np.asarray(Wo, dtype=np.float32)
    pos = np.asarray(position_ids).reshape(S).astype(np.float32)

    half = D // 2
    inv_freq = 1.0 / (THETA ** (np.arange(half, dtype=np.float32) * 2.0 / D))
    freqs = inv_freq[:, None] * pos[None, :]          # [64, S]
    c64 = np.cos(freqs, dtype=np.float32)
    s64 = np.sin(freqs, dtype=np.float32)
    cosT = np.vstack([c64, c64]).astype(NPBF16)       # [128, S]
    sinT = np.vstack([-s64, s64]).astype(NPBF16)      # signed for rotate-half
    tri = np.triu(np.ones((128, 128), dtype=np.float32)).astype(NPBF16)
    Wo_bf = wshuf(Wo)
    ident = np.eye(128, dtype=np.float32).astype(NPBF16)

    in_maps = []
    for c in range(NCORES):
        kvh = c // 2
        in_maps.append({
            "xT": xT,
            "wq": wshuf(Wq[:, c * HPC * D:(c + 1) * HPC * D]),
            "wk": wshuf(Wk[:, kvh * D:(kvh + 1) * D]),
            "wv": wshuf(Wv[:, kvh * D:(kvh + 1) * D]),
            "wo": Wo_bf,
            "cosT": cosT,
            "sinT": sinT,
            "trimask": tri,
            "identity": ident,
        })
    return in_maps


def _run(inputs, trace=False):
    nc = _get_nc()
    in_maps = _prep_inputs(**inputs)
    res = run_bass_kernel_spmd(nc, in_maps, list(range(NCORES)), trace=trace)
    full = np.empty((S, H), dtype=np.float32)
    for c in range(NCORES):
        shard = res.results[c]["out"]          # [NCH*128, H]
        for i in range(NCH):
            full[i * CHUNK + c * 128: i * CHUNK + (c + 1) * 128, :] = \
                shard[i * 128:(i + 1) * 128, :]
    return full.reshape(B, S, H), res


def kernel(**inputs):
    full, _ = _run(inputs, trace=False)
    return full


# revision 19
# speedup vs baseline: 1.0242x; 1.0242x over previous
"""Distributed Trainium2 kernel for DeepseekV3-style GQA attention.

Problem (hardcoded): B=1, S=4096, H=2048, NQ=16 q heads, NKV=4 kv heads,
D=128, rotate-half RoPE (theta=10000) over full head dim, causal softmax,
o_proj. 8 NeuronCores, tensor-parallel over heads:

  core c: q heads {2c, 2c+1}, kv head c//2 (replicated across the pair),
  Wq/Wk/Wv column-sharded, attention computed flash-style in bf16 with
  f32 PSUM accumulation, attention output produced transposed [j, s],
  AllGathered in 4 sequence chunks (overlapped with attention compute),
  o_proj column-sharded (Wo[:, c*256:(c+1)*256]) -> outT shard [256, S].

Host side: casts inputs to bf16, pre-transposes x, builds cos/sin tables
from position_ids, shards weights, transposes + concatenates out shards.
"""
import os
import sys

sys.path.insert(0, "/opt/trn_rl_repo")

import numpy as np
import ml_dtypes

import concourse.bass as bass
import concourse.bacc as bacc
import concourse.mybir as mybir
import concourse.tile as tile
from concourse.bass_utils import run_bass_kernel_spmd

BF16 = mybir.dt.bfloat16
F32 = mybir.dt.float32
NPBF16 = ml_dtypes.bfloat16

B, S, H = 1, 4096, 2048
NQ, NKV, D = 16, 4, 128
THETA = 10000.0
NCORES = 8
HPC = NQ // NCORES          # q heads per core = 2
OC = H // NCORES            # o_proj out cols per core = 256
SC = 512                    # projection s-chunk
NSC = S // SC               # 8
NKT = S // 128              # 32 k tiles of 128
QS = 512                    # attention q supertile
NQS = S // QS               # 8
CHUNK = 1024                # allgather s-chunk
NCH = S // CHUNK            # 4
SCALE = 1.0 / float(np.sqrt(D))

_cached = {}


def _build():
    nc = bacc.Bacc("TRN2", target_bir_lowering=False, debug=False,
                   num_devices=NCORES)

    xT = nc.declare_dram_parameter("xT", [NSC, 128, 16 * SC], BF16, isOutput=False)
    wq = nc.declare_dram_parameter("wq", [128, 16 * HPC * D], BF16, isOutput=False)
    wk = nc.declare_dram_parameter("wk", [128, 16 * D], BF16, isOutput=False)
    wv = nc.declare_dram_parameter("wv", [128, 16 * D], BF16, isOutput=False)
    wo = nc.declare_dram_parameter("wo", [128, 16 * H], BF16, isOutput=False)
    cosT = nc.declare_dram_parameter("cosT", [D, S], BF16, isOutput=False)
    sinT = nc.declare_dram_parameter("sinT", [D, S], BF16, isOutput=False)
    trimask = nc.declare_dram_parameter("trimask", [128, 128], BF16, isOutput=False)
    identity = nc.declare_dram_parameter("identity", [128, 128], BF16, isOutput=False)
    out = nc.declare_dram_parameter("out", [NCH * 128, H], F32, isOutput=True)

    with tile.TileContext(nc) as tc:
        with (
            tc.tile_pool(name="const", bufs=1) as constp,
            tc.tile_pool(name="persist", bufs=1) as persist,
            tc.tile_pool(name="xtp", bufs=2) as xtp,
            tc.tile_pool(name="ropep", bufs=2) as ropep,
            tc.tile_pool(name="ptp", bufs=3) as ptp,
            tc.tile_pool(name="attnp", bufs=3) as attnp,
            tc.tile_pool(name="smallp", bufs=3) as smallp,
            tc.tile_pool(name="agp", bufs=2) as agp,
            tc.tile_pool(name="outp", bufs=1) as outp,
            tc.tile_pool(name="dram", bufs=1, space="DRAM") as dramp,
        ):
            # ---- weights first (first matmuls need wq + xt chunk 0) ----
            wq_all = persist.tile([128, 16 * HPC * D], BF16, tag="wq")
            nc.sync.dma_start(wq_all[:], wq[:])
            wk_all = persist.tile([128, 16 * D], BF16, tag="wk")
            nc.sync.dma_start(wk_all[:], wk[:])
            wv_all = persist.tile([128, 16 * D], BF16, tag="wv")
            nc.sync.dma_start(wv_all[:], wv[:])

            cos_sb = persist.tile([128, S], BF16, tag="cos")
            sin_sb = persist.tile([128, S], BF16, tag="sin")
            tri = constp.tile([128, 128], BF16, tag="tri")
            ident = constp.tile([128, 128], BF16, tag="ident")
            wo_all = persist.tile([128, 16 * H], BF16, tag="wo")

            def wqt(t, h):
                return wq_all[:, t * HPC * D + h * D: t * HPC * D + (h + 1) * D]

            QT = [persist.tile([128, S], BF16, tag=f"qt{h}", name=f"qt{h}")
                  for h in range(HPC)]
            KT = persist.tile([128, S], BF16, tag="kt")
            V = []
            for kt in range(NKT):
                vt = persist.tile([128, D + 1], BF16, tag=f"v{kt}")
                nc.gpsimd.memset(vt[:, D:D + 1], 1.0)
                V.append(vt)

            # tiny barrier collective: absorbs the ~50us core-launch skew
            # during the input-DMA phase so the first real A2A doesn't pay it
            bar_in = dramp.tile([1, 128], F32, tag="bar_in", name="bar_in")
            bar_out = dramp.tile([1, 128], F32, tag="bar_out", name="bar_out",
                                 addr_space="Shared")
            nc.gpsimd.collective_compute(
                "AllReduce", mybir.AluOpType.add,
                replica_groups=[list(range(NCORES))],
                ins=[bar_in.opt()], outs=[bar_out.opt()])

            # ---- phase 1: projections (chunked over s) + RoPE ----
            with (
                tc.tile_pool(name="projps", bufs=4, space="PSUM") as projps,
                tc.tile_pool(name="vps", bufs=2, space="PSUM") as vps,
            ):
                for sc in range(NSC):
                    off = sc * SC
                    xt_all = xtp.tile([128, 16 * SC], BF16, tag="xt",
                                      name="xt_all")
                    nc.sync.dma_start(xt_all[:], xT[sc])
                    if sc == 0:
                        nc.sync.dma_start(cos_sb[:], cosT[:])
                        nc.sync.dma_start(sin_sb[:], sinT[:])
                        nc.sync.dma_start(tri[:], trimask[:])
                        nc.sync.dma_start(ident[:], identity[:])

                    def xts(t):
                        return xt_all[:, t * SC:(t + 1) * SC]

                    # q/k projections -> transposed layout [d, s]
                    def project_rope(lhs_of_t, dst):
                        ps = projps.tile([128, SC], F32, tag="proj", name="ps")
                        for t in range(16):
                            nc.tensor.matmul(ps[:], lhs_of_t(t), xts(t),
                                             start=(t == 0), stop=(t == 15))
                        raw = ropep.tile([128, SC], BF16, tag="raw", name="raw")
                        nc.vector.tensor_copy(raw[:], ps[:])
                        sw = ropep.tile([128, SC], BF16, tag="sw", name="sw")
                        nc.sync.dma_start(sw[0:64, :], raw[64:128, :])
                        nc.sync.dma_start(sw[64:128, :], raw[0:64, :])
                        t1 = ropep.tile([128, SC], BF16, tag="t1", name="t1")
                        nc.vector.tensor_tensor(t1[:], raw[:],
                                                cos_sb[:, off:off + SC],
                                                mybir.AluOpType.mult)
                        t2 = ropep.tile([128, SC], BF16, tag="t2", name="t2")
                        nc.vector.tensor_tensor(t2[:], sw[:],
                                                sin_sb[:, off:off + SC],
                                                mybir.AluOpType.mult)
                        nc.vector.tensor_tensor(dst[:, off:off + SC], t1[:],
                                                t2[:], mybir.AluOpType.add)

                    for h in range(HPC):
                        project_rope(lambda t, h=h: wqt(t, h), QT[h])
                    project_rope(lambda t: wk_all[:, t * D:(t + 1) * D], KT)

                    # v projection (natural [s, d] layout)
                    for st in range(SC // 128):
                        v_ps = vps.tile([128, 128], F32, tag="vps", name="v_ps")
                        for t in range(16):
                            nc.tensor.matmul(
                                v_ps[:], xt_all[:, t * SC + st * 128:
                                                t * SC + (st + 1) * 128],
                                wv_all[:, t * D:(t + 1) * D],
                                start=(t == 0), stop=(t == 15))
                        kti = sc * (SC // 128) + st
                        nc.vector.tensor_copy(V[kti][:, 0:D], v_ps[:])

            # wo is first needed mid-attention; load it after phase 1 so it
            # doesn't compete with the startup-critical DMAs
            nc.scalar.dma_start(wo_all[:], wo[:])

            # ---- phases 2+3 interleaved: attention, AG, o_proj ----
            with (
                tc.tile_pool(name="stps", bufs=2, space="PSUM") as stps,
                tc.tile_pool(name="attps", bufs=4, space="PSUM") as attps,
                tc.tile_pool(name="smps", bufs=2, space="PSUM") as smps,
            ):
                attnT_cur = [None, None]
                bounces = []

                def _attv(kt, j, q_lo, pt, att, qs):
                    for qsub in range(max(j, 0), 4):
                        nc.tensor.matmul(
                            att[qsub][:],
                            pt[:, qsub * 128 - q_lo:qsub * 128 - q_lo + 128],
                            V[kt][:],
                            start=(kt == 0), stop=(kt == 4 * qs + qsub))

                def attention(head, qs):
                    q_off = qs * QS
                    att = [attps.tile([128, D + 1], F32, tag="att", name=f"att{i}")
                           for i in range(4)]
                    nkt = 4 * qs + 4
                    pend = None  # (kt, j, q_lo, pt)
                    for kt in range(nkt):
                        j = kt - 4 * qs
                        q_lo = 128 * j if j > 0 else 0
                        N = QS - q_lo
                        st_ps = stps.tile([128, QS], F32, tag="st", name="st_ps")
                        nc.tensor.matmul(
                            st_ps[:, 0:N],
                            KT[:, kt * 128:(kt + 1) * 128],
                            QT[head][:, q_off + q_lo:q_off + QS],
                            start=True, stop=True)
                        pt = ptp.tile([128, QS], BF16, tag="pt", name="pt")
                        nc.scalar.activation(pt[:, 0:N], st_ps[:, 0:N],
                                             mybir.ActivationFunctionType.Exp,
                                             scale=SCALE)
                        if j >= 0:
                            nc.vector.tensor_tensor(pt[:, 0:128], pt[:, 0:128],
                                                    tri[:],
                                                    mybir.AluOpType.mult)
                        if pend is not None:
                            _attv(*pend, att, qs)
                        pend = (kt, j, q_lo, pt)
                    _attv(*pend, att, qs)

                    # normalize + transpose into attnT
                    for qsub in range(4):
                        recip = smallp.tile([128, 1], F32, tag="recip",
                                            name="recip")
                        nc.vector.reciprocal(recip[:], att[qsub][:, D:D + 1])
                        attn_n = smallp.tile([128, 128], BF16, tag="attn_n",
                                             name="attn_n")
                        nc.vector.tensor_scalar(attn_n[:], att[qsub][:, 0:D],
                                                recip[:], None,
                                                mybir.AluOpType.mult)
                        tr = smps.tile([128, 128], BF16, tag="tr", name="tr")
                        nc.tensor.transpose(tr[:], attn_n[:], ident[:])
                        col = (qs % 2) * QS + qsub * 128
                        nc.vector.tensor_copy(
                            attnT_cur[head][:, col:col + 128], tr[:])

                def emit_a2a(ci, bounce):
                    ex = dramp.tile([NCORES * 2 * 128, 128], BF16,
                                    tag=f"a2a{ci}", name=f"a2a{ci}")
                    nc.gpsimd.collective_compute(
                        "AllToAll", mybir.AluOpType.bypass,
                        replica_groups=[list(range(NCORES))],
                        ins=[bounce.opt()], outs=[ex.opt()])
                    bounces.append(ex)

                def emit_oproj(ci):
                    ex = bounces[ci]
                    ag_all = agp.tile([128, 16 * 128], BF16, tag="ag",
                                      name="ag_all")
                    nc.scalar.dma_start(
                        ag_all[:].rearrange("p (t s) -> p t s", t=16),
                        ex[:].rearrange("(t p) s -> p t s", p=128))
                    o_sb = outp.tile([128, H], F32, tag="osb", name="o_sb")
                    for ocg in range(H // 512):
                        o_ps = smps.tile([128, 512], F32, tag="tr",
                                         name="o_ps")
                        for jt in range(16):
                            nc.tensor.matmul(
                                o_ps[:],
                                ag_all[:, jt * 128:(jt + 1) * 128],
                                wo_all[:, jt * H + ocg * 512:
                                       jt * H + (ocg + 1) * 512],
                                start=(jt == 0), stop=(jt == 15))
                        nc.vector.tensor_copy(
                            o_sb[:, ocg * 512:(ocg + 1) * 512], o_ps[:])
                    nc.scalar.dma_start(out[ci * 128:(ci + 1) * 128, :],
                                        o_sb[:])

                bounce_cur = None
                for qs in range(NQS):
                    ci = qs // 2
                    if qs % 2 == 0:
                        bounce_cur = dramp.tile([NCORES * 2 * 128, 128], BF16,
                                                tag=f"bn{ci}", name=f"bn{ci}")
                        for head in range(HPC):
                            attnT_cur[head] = attnp.tile(
                                [128, CHUNK], BF16, tag=f"attnT{head}",
                                name=f"attnT{head}_{qs}")
                    for head in range(HPC):
                        attention(head, qs)
                    if qs % 2 == 1:
                        # scatter this chunk's attnT into the A2A bounce:
                        # dest core d gets s cols d*128..(d+1)*128 of the chunk
                        bv = bounce_cur[:].rearrange(
                            "(d h j) s -> h j d s", d=NCORES, h=HPC)
                        for head in range(HPC):
                            nc.scalar.dma_start(
                                bv[head],
                                attnT_cur[head][:].rearrange(
                                    "j (d s) -> j d s", d=NCORES))
                        emit_a2a(ci, bounce_cur)
                    # consume each chunk's exchange late to hide collective
                    # latency
                for ci in range(NCH):
                    emit_oproj(ci)

    nc.compile()
    return nc


def _get_nc():
    if "nc" not in _cached:
        _cached["nc"] = _build()
    return _cached["nc"]


def _prep_inputs(hidden_states, Wq, Wk, Wv, Wo, position_ids):
    x = np.asarray(hidden_states, dtype=np.float32).reshape(S, H)
    xT = np.ascontiguousarray(
        x.T.reshape(16, 128, NSC, SC).transpose(2, 1, 0, 3)
        .reshape(NSC, 128, 16 * SC)).astype(NPBF16)

    def wshuf(W):
        n = W.shape[1]
        return np.ascontiguousarray(
            W.reshape(16, 128, n).transpose(1, 0, 2).reshape(128, 16 * n)
        ).astype(NPBF16)
    Wq = np.asarray(Wq, dtype=np.float32)
    Wk = np.asarray(Wk, dtype=np.float32)
    Wv = np.asarray(Wv, dtype=np.float32)
    Wo = np.asarray(Wo, dtype=np.float32)
    pos = np.asarray(position_ids).reshape(S).astype(np.float32)

    half = D // 2
    inv_freq = 1.0 / (THETA ** (np.arange(half, dtype=np.float32) * 2.0 / D))
    freqs = inv_freq[:, None] * pos[None, :]          # [64, S]
    c64 = np.cos(freqs, dtype=np.float32)
    s64 = np.sin(freqs, dtype=np.float32)
    cosT = np.vstack([c64, c64]).astype(NPBF16)       # [128, S]
    sinT = np.vstack([-s64, s64]).astype(NPBF16)      # signed for rotate-half
    tri = np.triu(np.ones((128, 128), dtype=np.float32)).astype(NPBF16)
    Wo_bf = wshuf(Wo)
    ident = np.eye(128, dtype=np.float32).astype(NPBF16)

    in_maps = []
    for c in range(NCORES):
        kvh = c // 2
        in_maps.append({
            "xT": xT,
            "wq": wshuf(Wq[:, c * HPC * D:(c + 1) * HPC * D]),
            "wk": wshuf(Wk[:, kvh * D:(kvh + 1) * D]),
            "wv": wshuf(Wv[:, kvh * D:(kvh + 1) * D]),
            "wo": Wo_bf,
            "cosT": cosT,
            "sinT": sinT,
            "trimask": tri,
            "identity": ident,
        })
    return in_maps


def _run(inputs, trace=False):
    nc = _get_nc()
    in_maps = _prep_inputs(**inputs)
    res = run_bass_kernel_spmd(nc, in_maps, list(range(NCORES)), trace=trace)
    full = np.empty((S, H), dtype=np.float32)
    for c in range(NCORES):
        shard = res.results[c]["out"]          # [NCH*128, H]
        for i in range(NCH):
            full[i * CHUNK + c * 128: i * CHUNK + (c + 1) * 128, :] = \
                shard[i * 128:(i + 1) * 128, :]
    return full.reshape(B, S, H), res


def kernel(**inputs):
    full, _ = _run(inputs, trace=False)
    return full
